# revision 7
# baseline (speedup 1.0000x reference)
"""Trainium2 Bass kernel for nn_LossFunction_40346922778857 (v2).

Computes: scatter-loss over x (256,128,768).
  x1 = x[::2], x2 = x[1::2]  (each (128,128,768))
  per half: within (D,D), between (D,D) scatter matrices, corr-normalized,
  loss = sum((w1-w2)^2) + sum((b1-b2)^2).

Device computes per-half Gram G = X^T X (upper-triangle 128-row blocks) in
fp8e4 DoubleRow; per-b row sums S are computed on host from the SAME fp8
data (keeps within/between consistent with the quantized Gram).  Host sums
the 8 cores' partials and finishes the O(D^2) algebra in float64.

v2 layout (per core):
  in  x{h}: [128, 8(td), 2(s), 768] fp8, row (td*256+s*128+p) feature f
  out o{h}: [128, 2688] bf16, regions
      [i0a 512][i0b 256][i1a 512][i1b 128][i2 512][i3 384][i4 256][i5 128]
      region (i, colrange): G rows 128i:128(i+1), cols below.
"""

import numpy as np

P = 128
D = 768
NB = 16          # b's per half per core
TD = 8           # double-k-tiles per half (each contracts 256 rows)
NCORES = 8
OW = 2688        # output cols per half

_STATE = {}
LAST = {}

# (name, block i, col offset in G row-block, width, out col offset)
REGIONS = [
    ("i0a", 0, 0,   512, 0),
    ("i0b", 0, 512, 256, 512),
    ("i1a", 1, 128, 512, 768),
    ("i1b", 1, 640, 128, 1280),
    ("i2",  2, 256, 512, 1408),
    ("i3",  3, 384, 384, 1920),
    ("i4",  4, 512, 256, 2304),
    ("i5",  5, 640, 128, 2560),
]  # c0 = absolute G column start

# PSUM bank map per half: region -> (bank tile idx, col offset in bank)
# h0 packs into 6 banks (b1 and b5 shared) so h1 starts with b6/b7 free;
# within each td, emission order guarantees the bank-sharing start=True
# owner (i0b before i1b, i4 before i5) writes first.
# h1 uses 8 distinct banks, each fully ordered after its h0 readers via
# AP overlap (512-wide regions land on h0's shared banks so the whole-bank
# clear of start=True is covered by WAR deps on both h0 casts).
BANKS = [
    {"i0a": (0, 0), "i0b": (1, 0), "i1a": (2, 0), "i1b": (1, 256),
     "i2": (3, 0), "i3": (4, 0), "i4": (5, 0), "i5": (5, 256)},
    {"i0a": (7, 0), "i0b": (6, 0), "i1a": (1, 0), "i1b": (0, 0),
     "i2": (5, 0), "i3": (2, 0), "i4": (3, 0), "i5": (4, 0)},
]

# drain emission order + engine (v=DVE, a=ACT): ordered so the banks h1
# needs earliest are freed first on each engine.
DRAINS = [("i0a", "v"), ("i0b", "a"), ("i1b", "a"), ("i1a", "a"),
          ("i2", "v"), ("i3", "a"), ("i4", "v"), ("i5", "v")]

# output DMAs per half: (out col start, out col end, regions covered)
OUT_CHUNKS = [
    (0, 768, ("i0a", "i0b")),
    (768, 1408, ("i1a", "i1b")),
    (1408, 2304, ("i2", "i3")),
    (2304, 2688, ("i4", "i5")),
]


def _build():
    import concourse.tile as tile
    from concourse import bacc, mybir

    # (walrus --enable-ldw-opt=true was tried to dedup the i0a/i0b and
    # i1a/i1b shared weight loads, but its standalone-InstLdweights output
    # crashes CoreV3 codegen in this compiler build.)
    nc = bacc.Bacc("TRN2", target_bir_lowering=False, debug=False,
                   num_devices=NCORES)

    fp8 = mybir.dt.float8e4
    xins = [nc.dram_tensor(f"x{h}", [P, TD, 2, D], fp8,
                           kind="ExternalInput").ap() for h in range(2)]
    outs = [nc.dram_tensor(f"o{h}", [P, OW], mybir.dt.bfloat16,
                           kind="ExternalOutput").ap() for h in range(2)]

    with tile.TileContext(nc) as tc:
        with tc.tile_pool(name="xp", bufs=2) as xp, \
             tc.tile_pool(name="wp", bufs=1) as wp, \
             tc.tile_pool(name="pp", bufs=8, space="PSUM") as pp, \
             tc.tile_pool(name="op", bufs=2) as op:
            xts = [xp.tile([P, TD, 2, D], fp8, tag="xt", name=f"xt{h}")
                   for h in range(2)]
            ots = [op.tile([P, OW], mybir.dt.bfloat16, tag="ot",
                           name=f"ot{h}") for h in range(2)]
            banks = [pp.tile([P, 512], mybir.dt.float32, tag="ps",
                             name=f"bank{b}") for b in range(8)]
            wt = wp.tile([P, 640], mybir.dt.float16, tag="wt")

            # input DMAs: (4 td x partition-band) chunks — 6144B contiguous
            # per-partition runs; both HWDGE rings carry part of every td
            # window so arrival tracks consumption order.  The sync ring
            # gets a bigger band (72/56) because the scalar ring starts
            # ~1.4us later.  (A third gpsimd/SWDGE queue was tried: ~4us
            # latency + descriptor-gen cost made everything worse.)
            for h in range(2):
                for tdr in (slice(0, 4), slice(4, 8)):
                    nc.sync.dma_start(out=xts[h][0:72, tdr, :, :],
                                      in_=xins[h][0:72, tdr, :, :])
                    nc.scalar.dma_start(out=xts[h][72:128, tdr, :, :],
                                        in_=xins[h][72:128, tdr, :, :])

            # PE warm-up on zeros while inputs stream: ~3.4us of activity
            # brings the HAM clock gate to 8/8 as real matmuls start.
            # memset on gpsimd: its stream is free at t=0.
            nc.gpsimd.memset(wt[:], 0.0)
            for _ in range(8):
                nc.tensor.matmul(banks[6][:, :512], wt[:, :128],
                                 wt[:, 128:640], start=True, stop=True)

            rdict = {r[0]: r for r in REGIONS}

            def mm(h, td, name, st, sp):
                xt = xts[h]
                (_n, i, c0, w, _oc) = rdict[name]
                b, boff = BANKS[h][name]
                nc.tensor.matmul(banks[b][:, boff:boff + w],
                                 xt[:, td, :, 128 * i:128 * i + 128],
                                 xt[:, td, :, c0:c0 + w],
                                 start=st, stop=sp, skip_group_check=True,
                                 perf_mode=mybir.MatmulPerfMode.DoubleRow)

            for h in range(2):
                bmap = BANKS[h]
                # td-major over td0-5 (streams with input arrival); the
                # bank-sharing first writer (h0) uses start=True at td0
                # (start clears the whole 2KB bank incl. the co-located
                # region).
                for td in range(TD - 2):
                    for (name, i, c0, w, _oc) in REGIONS:
                        mm(h, td, name, (td == 0) and bmap[name][1] == 0,
                           False)
                # region-major over the last two tds: each region's
                # accumulation finishes ~one region apart, so casts and
                # output DMAs pipeline with the compute tail instead of
                # bunching after it.
                for name, _e in DRAINS:
                    mm(h, TD - 2, name, False, False)
                    mm(h, TD - 1, name, False, True)
                # drains: PSUM -> SBUF bf16 split across DVE + ACT
                ot = ots[h]
                for name, e in DRAINS:
                    (_n, i, c0, w, oc) = rdict[name]
                    b, boff = bmap[name]
                    src = banks[b][:, boff:boff + w]
                    dst = ot[:, oc:oc + w]
                    if e == "v":
                        nc.vector.tensor_copy(dst, src)
                    else:
                        nc.scalar.copy(dst, src)
                # outputs stream per chunk; h0 on the scalar ring, h1
                # split across both rings so its tail DMAs parallelize.
                for ci, (a, bnd, _regs) in enumerate(OUT_CHUNKS):
                    if h == 0:
                        oeng = nc.scalar
                    else:
                        oeng = nc.sync if ci % 2 == 0 else nc.scalar
                    oeng.dma_start(out=outs[h][:, a:bnd],
                                   in_=ot[:, a:bnd])
    nc.compile()
    return nc


def _get_nc():
    if "nc" not in _STATE:
        _STATE["nc"] = _build()
    return _STATE["nc"]


def _quantize(x):
    import ml_dtypes
    return x.astype(np.float16).astype(ml_dtypes.float8_e4m3)


def _prep_half(x8h):
    """x8h: (128, 128, 768) fp8 for one half -> per-core [128, 8, 2, 768]."""
    out = []
    for c in range(NCORES):
        blk = x8h[NB * c:NB * (c + 1)]                    # (16, 128, 768)
        # b = 2*td + s, n = p  ->  (p, td, s, f)
        out.append(np.ascontiguousarray(
            blk.reshape(TD, 2, P, D).transpose(2, 0, 1, 3)))
    return out


def kernel(x, label=None, genre_label=None, _trace=False):
    from concourse.bass_utils import run_bass_kernel_spmd

    nc = _get_nc()

    x = np.asarray(x, dtype=np.float32)
    x8 = [_quantize(x[0::2]), _quantize(x[1::2])]
    halves = [_prep_half(x8[0]), _prep_half(x8[1])]
    in_maps = [{"x0": halves[0][c], "x1": halves[1][c]} for c in range(NCORES)]

    # First execution of a freshly compiled NEFF has been observed to be
    # flaky (garbage output or device error); validate and retry.
    res = None
    for attempt in range(3):
        try:
            res = run_bass_kernel_spmd(nc, in_maps, list(range(NCORES)),
                                       trace=_trace)
        except Exception:
            if attempt == 2:
                raise
            continue
        ok = all(
            np.isfinite(np.asarray(res.results[c][f"o{h}"],
                                   dtype=np.float32)).all()
            and np.any(np.asarray(res.results[c][f"o{h}"], dtype=np.float32))
            for c in range(NCORES) for h in range(2))
        if ok:
            break
    LAST["res"] = res

    B = x.shape[0] // 2
    N = x.shape[1]
    tol = B * N

    loss = 0.0
    for h in range(2):
        U = np.zeros((D, D), dtype=np.float64)
        for c in range(NCORES):
            o = np.asarray(res.results[c][f"o{h}"], dtype=np.float64)
            for (name, i, c0, w, oc) in REGIONS:
                U[128 * i:128 * (i + 1), c0:c0 + w] += o[:, oc:oc + w]
        G = np.zeros((D, D), dtype=np.float64)
        for i in range(6):
            ri = slice(P * i, P * (i + 1))
            G[ri, P * i:D] = U[ri, P * i:D]
            for j in range(i + 1, 6):
                rj = slice(P * j, P * (j + 1))
                G[rj, ri] = U[ri, rj].T
        # row sums from the same quantized data (consistent with G)
        S = x8[h].astype(np.float64).sum(axis=1)          # (B, D)
        xbar = S / N
        M = xbar.T @ xbar
        mean = xbar.mean(axis=0)
        within = (G - N * M) / tol
        between = N * (M - B * np.outer(mean, mean)) / tol
        w_h = within / np.sqrt(np.sum(np.diagonal(within) ** 2))
        b_h = between / np.sqrt(np.sum(np.diagonal(between) ** 2))
        if h == 0:
            w0, b0 = w_h, b_h
        else:
            loss = np.sum((w0 - w_h) ** 2) + np.sum((b0 - b_h) ** 2)
    return np.asarray(loss, dtype=np.float32)


# revision 8
# speedup vs baseline: 1.0835x; 1.0835x over previous
"""Trainium2 Bass kernel for nn_LossFunction_40346922778857.

Computes: scatter-loss over x (256,128,768).
  x1 = x[::2], x2 = x[1::2]  (each (128,128,768))
  per half: within (D,D), between (D,D) scatter matrices, corr-normalized,
  loss = sum((w1-w2)^2) + sum((b1-b2)^2).

Strategy (data-parallel over b across 8 cores):
  within = (G - N * Xbar^T Xbar) / (B*N)   with G = X^T X over (B*N, D)
  between = N * (Xbar^T Xbar - B mean mean^T) / (B*N)
  Each core computes partial G (upper-triangle 128-row blocks only; fp8e4
  inputs with DoubleRow 2x tensor-engine packing, fp32 PSUM accumulation)
  for its 16 even + 16 odd b's.  Per-b row-sums S fall out of the same
  matmuls via 16 appended one-hot columns.  Host sums the 8 partial
  results and finishes the O(D^2) algebra in float64.
  Measured: ~37.5 us HW exec, rel err ~1.3e-5 vs fp32 reference.
"""

import numpy as np

P = 128          # partitions / rows per b
D = 768          # feature dim
NB = 16          # number of b's (tiles) per half per core
DA = D + NB      # augmented width (one-hot tile-index columns)
L = 4            # k-tiles per DMA quarter
NQ = NB // L     # quarters per half
NCORES = 8
NBLK = D // P    # 6 row blocks of G

_STATE = {}
LAST = {}
FP8 = True     # fp8e4 + DoubleRow tensor-engine path (rel err ~1e-4 vs ~5e-7 fp16)
ND = NB // 2   # double-k-tiles per half per core (DoubleRow contracts 256 rows)


def _chunks_for(w_all):
    chunks = []
    off = 0
    while off < w_all:
        w = min(512, w_all - off)
        chunks.append((off, w))
        off += w
    return chunks


def _build():
    import concourse.tile as tile
    from concourse import bacc, mybir
    from concourse.tile import add_dep_helper

    nc = bacc.Bacc("TRN2", target_bir_lowering=False, debug=False,
                   num_devices=NCORES)

    in_dt = mybir.dt.float8e4 if FP8 else mybir.dt.float16
    # fp8: quarter = 2 double-k-tiles, free layout (dt2, j, f); fp16: 4 k-tiles
    xins = [nc.dram_tensor(f"x{h}", [NQ, P, L * DA], in_dt,
                           kind="ExternalInput").ap() for h in range(2)]
    outs = [nc.dram_tensor(f"o{h}", [D, DA], mybir.dt.bfloat16,
                           kind="ExternalOutput").ap() for h in range(2)]

    with tile.TileContext(nc) as tc:
        with tc.tile_pool(name="xp", bufs=2 * NQ) as xp, \
             tc.tile_pool(name="wp", bufs=1) as wp, \
             tc.tile_pool(name="pp", bufs=6, space="PSUM") as pp, \
             tc.tile_pool(name="wpp", bufs=1, space="PSUM") as wpp, \
             tc.tile_pool(name="op", bufs=6) as op:
            # PE warm-up: dummy matmuls while input DMAs stream, so the HAM
            # clock gate is at 8/8 when real matmuls start.
            wt = wp.tile([P, P], mybir.dt.float16, tag="wt")
            nc.vector.memset(wt[:], 0.0)
            wps = wpp.tile([P, P], mybir.dt.float32, tag="wps")
            for _ in range(22):
                nc.tensor.matmul(wps[:], wt[:], wt[:], start=True, stop=True)

            # Input DMAs, chained depth-2 so tiles arrive roughly in
            # consumption order instead of 8-way fair sharing.
            dma_chain = []
            all_q_tiles = [[], []]
            for h in range(2):
                xin = xins[h]
                for q in range(NQ):
                    if FP8:
                        xt = xp.tile([P, 2, 2, DA], in_dt, tag="xt",
                                     name=f"x{h}q{q}")
                        xr = xin[q].rearrange("p (a b f) -> p a b f", a=2, b=2)
                        d = nc.sync.dma_start(out=xt[:], in_=xr)
                    else:
                        xt = xp.tile([P, L * DA], in_dt, tag="xt",
                                     name=f"x{h}q{q}")
                        d = nc.sync.dma_start(out=xt[:], in_=xin[q])
                    if len(dma_chain) >= 2:
                        add_dep_helper(d.ins, dma_chain[-2].ins,
                                       reason="input dma ordering")
                    dma_chain.append(d)
                    all_q_tiles[h].append(xt)

            last_in = dma_chain[-1]
            # Two sweeps of row-blocks per half, k-tile-outer within a sweep:
            # one arrived quarter unlocks ALL its row-block matmuls (~2.9us of
            # PE work per quarter vs ~1.2us stream time -> no input starvation).
            for h in range(2):
                oout = outs[h]
                q_tiles = all_q_tiles[h]
                # h0 first sweep is k-outer across 3 row-blocks (input still
                # streaming); once data is resident, per-block passes retire
                # PSUM + outputs sooner.
                sweeps = (((0, 1, 2), (3,), (4,), (5,)) if h == 0 else
                          ((0,), (1,), (2,), (3,), (4,), (5,)))
                for sweep in sweeps:
                    pts = {}
                    for i in sweep:
                        for ci in range(len(_chunks_for(DA - P * i))):
                            pts[i, ci] = pp.tile([P, 512], mybir.dt.float32,
                                                 tag="ps", name=f"ps{h}b{i}c{ci}")
                    if FP8:
                        for td in range(ND):
                            q, dt2 = divmod(td, 2)
                            xt = q_tiles[q]
                            for i in sweep:
                                c0 = P * i
                                lhsT = xt[:, dt2, :, c0:c0 + P]
                                for ci, (off, w) in enumerate(_chunks_for(DA - c0)):
                                    nc.tensor.matmul(
                                        pts[i, ci][:, :w], lhsT,
                                        xt[:, dt2, :, c0 + off:c0 + off + w],
                                        start=(td == 0), stop=(td == ND - 1),
                                        perf_mode=mybir.MatmulPerfMode.DoubleRow)
                    else:
                        for t in range(NB):
                            q, l = divmod(t, L)
                            xt = q_tiles[q]
                            for i in sweep:
                                base = l * DA + P * i
                                lhsT = xt[:, base:base + P]
                                for ci, (off, w) in enumerate(_chunks_for(DA - P * i)):
                                    nc.tensor.matmul(
                                        pts[i, ci][:, :w], lhsT,
                                        xt[:, base + off:base + off + w],
                                        start=(t == 0), stop=(t == NB - 1))
                    for i in sweep:
                        w_all = DA - P * i
                        ot = op.tile([P, w_all], mybir.dt.bfloat16, tag="ot",
                                     name=f"o{h}b{i}")
                        for ci, (off, w) in enumerate(_chunks_for(w_all)):
                            nc.vector.tensor_copy(ot[:, off:off + w],
                                                  pts[i, ci][:, :w])
                        # scalar-engine HWDGE ring + gated behind input so
                        # output traffic never steals input bandwidth
                        dout = nc.scalar.dma_start(
                            out=oout[P * i:P * (i + 1), P * i:DA], in_=ot[:])
                        add_dep_helper(dout.ins, last_in.ins,
                                       reason="outputs after inputs")
    nc.compile()
    return nc


def _get_nc():
    if "nc" not in _STATE:
        _STATE["nc"] = _build()
    return _STATE["nc"]


def _prep_half(xh):
    """xh: (128, 128, 768) f32 for one half -> per-core list of (NQ,P,L*DA)."""
    out = []
    for c in range(NCORES):
        blk = xh[NB * c:NB * (c + 1)]                      # (16, 128, 768)
        arr = np.zeros((NB, P, DA), dtype=np.float16)
        arr[:, :, :D] = blk
        for j in range(NB):
            arr[j, :, D + j] = 1.0
        if FP8:
            import ml_dtypes
            arr8 = arr.astype(ml_dtypes.float8_e4m3)
            # t = 4q + 2*dt2 + j -> (q, p, dt2, j, f)
            out.append(np.ascontiguousarray(
                arr8.reshape(NQ, 2, 2, P, DA).transpose(0, 3, 1, 2, 4)
                    .reshape(NQ, P, L * DA)))
        else:
            # (t=4q+l, p, f) -> (q, p, l*DA+f)
            out.append(np.ascontiguousarray(
                arr.reshape(NQ, L, P, DA).transpose(0, 2, 1, 3)
                   .reshape(NQ, P, L * DA)))
    return out


def kernel(x, label=None, genre_label=None, _trace=False):
    from concourse.bass_utils import run_bass_kernel_spmd

    nc = _get_nc()

    x = np.asarray(x, dtype=np.float32)
    halves = [_prep_half(x[0::2]), _prep_half(x[1::2])]
    in_maps = [{"x0": halves[0][c], "x1": halves[1][c]} for c in range(NCORES)]

    # First execution of a freshly compiled NEFF has been observed to be
    # flaky (garbage output or device error); validate and retry.
    res = None
    for attempt in range(3):
        try:
            res = run_bass_kernel_spmd(nc, in_maps, list(range(NCORES)),
                                       trace=_trace)
        except Exception:
            if attempt == 2:
                raise
            continue
        ok = all(
            np.isfinite(np.asarray(res.results[c][f"o{h}"],
                                   dtype=np.float32)).all()
            and np.any(np.asarray(res.results[c][f"o{h}"], dtype=np.float32))
            for c in range(NCORES) for h in range(2))
        if ok:
            break
    LAST["res"] = res

    B = x.shape[0] // 2          # 128 b's per half
    N = x.shape[1]               # 128 rows per b
    tol = B * N

    loss = 0.0
    for h in range(2):
        U = np.zeros((D, D), dtype=np.float64)
        S = np.zeros((B, D), dtype=np.float64)
        for c in range(NCORES):
            o = np.asarray(res.results[c][f"o{h}"], dtype=np.float64)
            for i in range(NBLK):
                r = slice(P * i, P * (i + 1))
                U[r, P * i:D] += o[r, P * i:D]
            S[NB * c:NB * (c + 1)] += o[:, D:DA].T
        G = np.zeros((D, D), dtype=np.float64)
        for i in range(NBLK):
            ri = slice(P * i, P * (i + 1))
            G[ri, ri] = U[ri, ri]
            for j in range(i + 1, NBLK):
                rj = slice(P * j, P * (j + 1))
                G[ri, rj] = U[ri, rj]
                G[rj, ri] = U[ri, rj].T
        xbar = S / N
        M = xbar.T @ xbar
        mean = xbar.mean(axis=0)
        within = (G - N * M) / tol
        between = N * (M - B * np.outer(mean, mean)) / tol
        w_h = within / np.sqrt(np.sum(np.diagonal(within) ** 2))
        b_h = between / np.sqrt(np.sum(np.diagonal(between) ** 2))
        if h == 0:
            w0, b0 = w_h, b_h
        else:
            loss = np.sum((w0 - w_h) ** 2) + np.sum((b0 - b_h) ** 2)
    return np.asarray(loss, dtype=np.float32)



# revision 10
# speedup vs baseline: 1.3473x; 1.2434x over previous
"""Trainium2 Bass kernel for nn_LossFunction_40346922778857 (v7).

Computes: scatter-loss over x (256,128,768).
  x1 = x[::2], x2 = x[1::2]  (each (128,128,768))
  per half: within (D,D), between (D,D) scatter matrices, corr-normalized,
  loss = sum((w1-w2)^2) + sum((b1-b2)^2).

Device computes per-half Gram G = X^T X (upper-triangle 128-row blocks) in
fp8e4 DoubleRow; per-b row sums S are computed on host from the SAME fp8
data (keeps within/between consistent with the quantized Gram).  Host sums
the 8 cores' partials and finishes the O(D^2) algebra in float64.

Layout (per core):
  in  x{h}: [128, 8(td), 2(s), 768] fp8, row (td*256+s*128+p) feature f
  out o{h}: [128, 2688] bf16, regions
      [i0a 512][i0b 256][i1a 512][i1b 128][i2 512][i3 384][i4 256][i5 128]
      region (i, colrange): G rows 128i:128(i+1), cols below.

Schedule: inputs stream as 2-td chunks alternating the two HWDGE rings
(sync/scalar) in consumption order; PE warms up on zeros then runs
td-major per half with the last two tds region-major so PSUM drains
(DVE+ACT split) and per-chunk output DMAs overlap the compute tail;
h1's bank map recycles h0's banks in drain order.
Measured (alternating A/B on one session): ~35.4us vs 38.6us for the
session baseline under identical conditions; rel err ~1.5e-6.
"""

import numpy as np

P = 128
D = 768
NB = 16          # b's per half per core
TD = 8           # double-k-tiles per half (each contracts 256 rows)
NCORES = 8
OW = 2688        # output cols per half

_STATE = {}
LAST = {}

# (name, block i, col offset in G row-block, width, out col offset)
REGIONS = [
    ("i0a", 0, 0,   512, 0),
    ("i0b", 0, 512, 256, 512),
    ("i1a", 1, 128, 512, 768),
    ("i1b", 1, 640, 128, 1280),
    ("i2",  2, 256, 512, 1408),
    ("i3",  3, 384, 384, 1920),
    ("i4",  4, 512, 256, 2304),
    ("i5",  5, 640, 128, 2560),
]  # c0 = absolute G column start

# PSUM bank map per half: region -> (bank tile idx, col offset in bank)
# h0 packs into 6 banks (b1 and b5 shared) so h1 starts with b6/b7 free;
# within each td, emission order guarantees the bank-sharing start=True
# owner (i0b before i1b, i4 before i5) writes first.
# h1 uses 8 distinct banks, each fully ordered after its h0 readers via
# AP overlap (512-wide regions land on h0's shared banks so the whole-bank
# clear of start=True is covered by WAR deps on both h0 casts).
BANKS = [
    {"i0a": (0, 0), "i0b": (1, 0), "i1a": (2, 0), "i1b": (1, 256),
     "i2": (3, 0), "i3": (4, 0), "i4": (5, 0), "i5": (5, 256)},
    {"i0a": (7, 0), "i0b": (6, 0), "i1a": (1, 0), "i1b": (0, 0),
     "i2": (5, 0), "i3": (2, 0), "i4": (3, 0), "i5": (4, 0)},
]

# drain emission order + engine (v=DVE, a=ACT): ordered so the banks h1
# needs earliest are freed first on each engine.
DRAINS = [("i0a", "v"), ("i0b", "a"), ("i1b", "a"), ("i1a", "a"),
          ("i2", "v"), ("i3", "a"), ("i4", "v"), ("i5", "v")]

# output DMAs per half: (out col start, out col end, regions covered)
OUT_CHUNKS = [
    (0, 768, ("i0a", "i0b")),
    (768, 1408, ("i1a", "i1b")),
    (1408, 2304, ("i2", "i3")),
    (2304, 2688, ("i4", "i5")),
]


def _build():
    import concourse.tile as tile
    from concourse import bacc, mybir

    # (walrus --enable-ldw-opt=true was tried to dedup the i0a/i0b and
    # i1a/i1b shared weight loads, but its standalone-InstLdweights output
    # crashes CoreV3 codegen in this compiler build.)
    nc = bacc.Bacc("TRN2", target_bir_lowering=False, debug=False,
                   num_devices=NCORES)

    fp8 = mybir.dt.float8e4
    xins = [nc.dram_tensor(f"x{h}", [P, TD, 2, D], fp8,
                           kind="ExternalInput").ap() for h in range(2)]
    outs = [nc.dram_tensor(f"o{h}", [P, OW], mybir.dt.bfloat16,
                           kind="ExternalOutput").ap() for h in range(2)]

    with tile.TileContext(nc) as tc:
        with tc.tile_pool(name="xp", bufs=2) as xp, \
             tc.tile_pool(name="wp", bufs=1) as wp, \
             tc.tile_pool(name="pp", bufs=8, space="PSUM") as pp, \
             tc.tile_pool(name="op", bufs=2) as op:
            xts = [xp.tile([P, TD, 2, D], fp8, tag="xt", name=f"xt{h}")
                   for h in range(2)]
            ots = [op.tile([P, OW], mybir.dt.bfloat16, tag="ot",
                           name=f"ot{h}") for h in range(2)]
            banks = [pp.tile([P, 512], mybir.dt.float32, tag="ps",
                             name=f"bank{b}") for b in range(8)]
            wt = wp.tile([P, 640], mybir.dt.float16, tag="wt")

            # input DMAs: (4 td x partition-band) chunks — 6144B contiguous
            # per-partition runs; both HWDGE rings carry part of every td
            # window so arrival tracks consumption order.  The sync ring
            # gets a bigger band (72/56) because the scalar ring starts
            # ~1.4us later.  (A third gpsimd/SWDGE queue was tried: ~4us
            # latency + descriptor-gen cost made everything worse.)
            for h in range(2):
                for eng, tdr in ((nc.sync, slice(0, 2)),
                                 (nc.scalar, slice(2, 4)),
                                 (nc.sync, slice(4, 6)),
                                 (nc.scalar, slice(6, 8))):
                    eng.dma_start(out=xts[h][:, tdr, :, :],
                                  in_=xins[h][:, tdr, :, :])

            # PE warm-up on zeros while inputs stream: ~3.4us of activity
            # brings the HAM clock gate to 8/8 as real matmuls start.
            # memset on gpsimd: its stream is free at t=0.
            nc.gpsimd.memset(wt[:], 0.0)
            for _ in range(8):
                nc.tensor.matmul(banks[6][:, :512], wt[:, :128],
                                 wt[:, 128:640], start=True, stop=True)

            rdict = {r[0]: r for r in REGIONS}

            def mm(h, td, name, st, sp):
                xt = xts[h]
                (_n, i, c0, w, _oc) = rdict[name]
                b, boff = BANKS[h][name]
                nc.tensor.matmul(banks[b][:, boff:boff + w],
                                 xt[:, td, :, 128 * i:128 * i + 128],
                                 xt[:, td, :, c0:c0 + w],
                                 start=st, stop=sp, skip_group_check=True,
                                 perf_mode=mybir.MatmulPerfMode.DoubleRow)

            for h in range(2):
                bmap = BANKS[h]
                # td-major over td0-5 (streams with input arrival); the
                # bank-sharing first writer (h0) uses start=True at td0
                # (start clears the whole 2KB bank incl. the co-located
                # region).
                for td in range(TD - 2):
                    for (name, i, c0, w, _oc) in REGIONS:
                        mm(h, td, name, (td == 0) and bmap[name][1] == 0,
                           False)
                # region-major over the last two tds: each region's
                # accumulation finishes ~one region apart, so casts and
                # output DMAs pipeline with the compute tail instead of
                # bunching after it.
                for name, _e in DRAINS:
                    mm(h, TD - 2, name, False, False)
                    mm(h, TD - 1, name, False, True)
                # drains: PSUM -> SBUF bf16 split across DVE + ACT
                ot = ots[h]
                for name, e in DRAINS:
                    (_n, i, c0, w, oc) = rdict[name]
                    b, boff = bmap[name]
                    src = banks[b][:, boff:boff + w]
                    dst = ot[:, oc:oc + w]
                    if e == "v":
                        nc.vector.tensor_copy(dst, src)
                    else:
                        nc.scalar.copy(dst, src)
                # outputs stream per chunk; h0 on the scalar ring, h1
                # split across both rings so its tail DMAs parallelize.
                for ci, (a, bnd, _regs) in enumerate(OUT_CHUNKS):
                    if h == 0:
                        oeng = nc.scalar
                    else:
                        oeng = nc.sync if ci % 2 == 0 else nc.scalar
                    oeng.dma_start(out=outs[h][:, a:bnd],
                                   in_=ot[:, a:bnd])
    nc.compile()
    return nc


def _get_nc():
    if "nc" not in _STATE:
        _STATE["nc"] = _build()
    return _STATE["nc"]


def _quantize(x):
    import ml_dtypes
    return x.astype(np.float16).astype(ml_dtypes.float8_e4m3)


def _prep_half(x8h):
    """x8h: (128, 128, 768) fp8 for one half -> per-core [128, 8, 2, 768]."""
    out = []
    for c in range(NCORES):
        blk = x8h[NB * c:NB * (c + 1)]                    # (16, 128, 768)
        # b = 2*td + s, n = p  ->  (p, td, s, f)
        out.append(np.ascontiguousarray(
            blk.reshape(TD, 2, P, D).transpose(2, 0, 1, 3)))
    return out


def kernel(x, label=None, genre_label=None, _trace=False):
    from concourse.bass_utils import run_bass_kernel_spmd

    nc = _get_nc()

    x = np.asarray(x, dtype=np.float32)
    x8 = [_quantize(x[0::2]), _quantize(x[1::2])]
    halves = [_prep_half(x8[0]), _prep_half(x8[1])]
    in_maps = [{"x0": halves[0][c], "x1": halves[1][c]} for c in range(NCORES)]

    # First execution of a freshly compiled NEFF has been observed to be
    # flaky (garbage output or device error); validate and retry.
    res = None
    for attempt in range(3):
        try:
            res = run_bass_kernel_spmd(nc, in_maps, list(range(NCORES)),
                                       trace=_trace)
        except Exception:
            if attempt == 2:
                raise
            continue
        ok = all(
            np.isfinite(np.asarray(res.results[c][f"o{h}"],
                                   dtype=np.float32)).all()
            and np.any(np.asarray(res.results[c][f"o{h}"], dtype=np.float32))
            for c in range(NCORES) for h in range(2))
        if ok:
            break
    LAST["res"] = res

    B = x.shape[0] // 2
    N = x.shape[1]
    tol = B * N

    loss = 0.0
    for h in range(2):
        U = np.zeros((D, D), dtype=np.float64)
        for c in range(NCORES):
            o = np.asarray(res.results[c][f"o{h}"], dtype=np.float64)
            for (name, i, c0, w, oc) in REGIONS:
                U[128 * i:128 * (i + 1), c0:c0 + w] += o[:, oc:oc + w]
        G = np.zeros((D, D), dtype=np.float64)
        for i in range(6):
            ri = slice(P * i, P * (i + 1))
            G[ri, P * i:D] = U[ri, P * i:D]
            for j in range(i + 1, 6):
                rj = slice(P * j, P * (j + 1))
                G[rj, ri] = U[ri, rj].T
        # row sums from the same quantized data (consistent with G)
        S = x8[h].astype(np.float64).sum(axis=1)          # (B, D)
        xbar = S / N
        M = xbar.T @ xbar
        mean = xbar.mean(axis=0)
        within = (G - N * M) / tol
        between = N * (M - B * np.outer(mean, mean)) / tol
        w_h = within / np.sqrt(np.sum(np.diagonal(within) ** 2))
        b_h = between / np.sqrt(np.sum(np.diagonal(between) ** 2))
        if h == 0:
            w0, b0 = w_h, b_h
        else:
            loss = np.sum((w0 - w_h) ** 2) + np.sum((b0 - b_h) ** 2)
    return np.asarray(loss, dtype=np.float32)


# revision 11
# speedup vs baseline: 1.3708x; 1.0175x over previous
"""Trainium2 Bass kernel for nn_LossFunction_40346922778857 (v12).

Computes: scatter-loss over x (256,128,768).
  x1 = x[::2], x2 = x[1::2]  (each (128,128,768))
  per half: within (D,D), between (D,D) scatter matrices, corr-normalized,
  loss = sum((w1-w2)^2) + sum((b1-b2)^2).

Device computes per-half Gram G = X^T X (upper-triangle 128-row blocks) in
fp8e4 DoubleRow; per-b row sums S are computed on host from the SAME fp8
data (keeps within/between consistent with the quantized Gram).  Host sums
the 8 cores' partials and finishes the O(D^2) algebra in float64.

Layout (per core):
  in  x{h}: [128, 8(td), 2(s), 768] fp8, row (td*256+s*128+p) feature f
  out o{h}: [128, 2688] bf16, regions
      [i0a 512][i0b 256][i1a 512][i1b 128][i2 512][i3 384][i4 256][i5 128]
      region (i, colrange): G rows 128i:128(i+1), cols below.
"""

import numpy as np

P = 128
D = 768
NB = 16          # b's per half per core
TD = 8           # double-k-tiles per half (each contracts 256 rows)
NCORES = 8
OW = 2688        # output cols per half

_STATE = {}
LAST = {}

# (name, block i, col offset in G row-block, width, out col offset)
REGIONS = [
    ("i0a", 0, 0,   512, 0),
    ("i0b", 0, 512, 256, 512),
    ("i1a", 1, 128, 512, 768),
    ("i1b", 1, 640, 128, 1280),
    ("i2",  2, 256, 512, 1408),
    ("i3",  3, 384, 384, 1920),
    ("i4",  4, 512, 256, 2304),
    ("i5",  5, 640, 128, 2560),
]  # c0 = absolute G column start

# PSUM bank map per half: region -> (bank tile idx, col offset in bank)
# h0 packs into 6 banks (b1 and b5 shared) so h1 starts with b6/b7 free;
# within each td, emission order guarantees the bank-sharing start=True
# owner (i0b before i1b, i4 before i5) writes first.
# h1 uses 8 distinct banks, each fully ordered after its h0 readers via
# AP overlap (512-wide regions land on h0's shared banks so the whole-bank
# clear of start=True is covered by WAR deps on both h0 casts).
BANKS = [
    {"i0a": (0, 0), "i0b": (1, 0), "i1a": (2, 0), "i1b": (1, 256),
     "i2": (3, 0), "i3": (4, 0), "i4": (5, 0), "i5": (5, 256)},
    {"i0a": (7, 0), "i0b": (6, 0), "i1a": (1, 0), "i1b": (0, 0),
     "i2": (5, 0), "i3": (2, 0), "i4": (3, 0), "i5": (4, 0)},
]

# drain emission order + engine (v=DVE, a=ACT): ordered so the banks h1
# needs earliest are freed first on each engine.
DRAINS = [("i0a", "v"), ("i0b", "a"), ("i1b", "a"), ("i1a", "a"),
          ("i2", "v"), ("i3", "a"), ("i4", "v"), ("i5", "v")]

# output DMAs per half: (out col start, out col end, regions covered)
OUT_CHUNKS = [
    (0, 768, ("i0a", "i0b")),
    (768, 1408, ("i1a", "i1b")),
    (1408, 2304, ("i2", "i3")),
    (2304, 2688, ("i4", "i5")),
]


def _build():
    import concourse.tile as tile
    from concourse import bacc, mybir

    # (walrus --enable-ldw-opt=true was tried to dedup the i0a/i0b and
    # i1a/i1b shared weight loads, but its standalone-InstLdweights output
    # crashes CoreV3 codegen in this compiler build.)
    nc = bacc.Bacc("TRN2", target_bir_lowering=False, debug=False,
                   num_devices=NCORES)

    fp8 = mybir.dt.float8e4
    xins = [nc.dram_tensor(f"x{h}", [P, TD, 2, D], fp8,
                           kind="ExternalInput").ap() for h in range(2)]
    outs = [nc.dram_tensor(f"o{h}", [P, OW], mybir.dt.bfloat16,
                           kind="ExternalOutput").ap() for h in range(2)]

    with tile.TileContext(nc) as tc:
        with tc.tile_pool(name="xp", bufs=2) as xp, \
             tc.tile_pool(name="wp", bufs=1) as wp, \
             tc.tile_pool(name="pp", bufs=8, space="PSUM") as pp, \
             tc.tile_pool(name="op", bufs=2) as op:
            xts = [xp.tile([P, TD, 2, D], fp8, tag="xt", name=f"xt{h}")
                   for h in range(2)]
            ots = [op.tile([P, OW], mybir.dt.bfloat16, tag="ot",
                           name=f"ot{h}") for h in range(2)]
            banks = [pp.tile([P, 512], mybir.dt.float32, tag="ps",
                             name=f"bank{b}") for b in range(8)]
            wt = wp.tile([P, 640], mybir.dt.float16, tag="wt")

            # input DMAs: (4 td x partition-band) chunks — 6144B contiguous
            # per-partition runs; both HWDGE rings carry part of every td
            # window so arrival tracks consumption order.  The sync ring
            # gets a bigger band (72/56) because the scalar ring starts
            # ~1.4us later.  (A third gpsimd/SWDGE queue was tried: ~4us
            # latency + descriptor-gen cost made everything worse.)
            for h in range(2):
                for eng, tdr in ((nc.sync, slice(0, 2)),
                                 (nc.scalar, slice(2, 4)),
                                 (nc.sync, slice(4, 6)),
                                 (nc.scalar, slice(6, 8))):
                    eng.dma_start(out=xts[h][:, tdr, :, :],
                                  in_=xins[h][:, tdr, :, :])

            # PE warm-up on zeros while inputs stream: ~3.4us of activity
            # brings the HAM clock gate to 8/8 as real matmuls start.
            # memset on gpsimd: its stream is free at t=0.
            nc.gpsimd.memset(wt[:], 0.0)
            for _ in range(8):
                nc.tensor.matmul(banks[6][:, :512], wt[:, :128],
                                 wt[:, 128:640], start=True, stop=True)

            rdict = {r[0]: r for r in REGIONS}

            def mm(h, td, name, st, sp):
                xt = xts[h]
                (_n, i, c0, w, _oc) = rdict[name]
                b, boff = BANKS[h][name]
                nc.tensor.matmul(banks[b][:, boff:boff + w],
                                 xt[:, td, :, 128 * i:128 * i + 128],
                                 xt[:, td, :, c0:c0 + w],
                                 start=st, stop=sp, skip_group_check=True,
                                 perf_mode=mybir.MatmulPerfMode.DoubleRow)

            for h in range(2):
                bmap = BANKS[h]
                # td-major over td0-5 (streams with input arrival); the
                # bank-sharing first writer (h0) uses start=True at td0
                # (start clears the whole 2KB bank incl. the co-located
                # region).
                for td in range(TD - 4):
                    for (name, i, c0, w, _oc) in REGIONS:
                        mm(h, td, name, (td == 0) and bmap[name][1] == 0,
                           False)
                # region-PAIR-major over the last four tds: pairs of
                # regions complete ~1.2us apart so PSUM drains and output
                # DMAs pipeline with the compute tail, while consecutive
                # matmuls alternate banks (no same-bank accumulate
                # serialization on the PE write port).
                dn = [n for n, _e in DRAINS]
                for pa, pb in ((dn[0], dn[1]), (dn[2], dn[3]),
                               (dn[4], dn[5]), (dn[6], dn[7])):
                    for td in range(TD - 4, TD):
                        mm(h, td, pa, False, td == TD - 1)
                        mm(h, td, pb, False, td == TD - 1)
                # drains: PSUM -> SBUF bf16 split across DVE + ACT
                ot = ots[h]
                for name, e in DRAINS:
                    (_n, i, c0, w, oc) = rdict[name]
                    b, boff = bmap[name]
                    src = banks[b][:, boff:boff + w]
                    dst = ot[:, oc:oc + w]
                    if e == "v":
                        nc.vector.tensor_copy(dst, src)
                    else:
                        nc.scalar.copy(dst, src)
                # outputs stream per chunk; h0 on the scalar ring, h1
                # split across both rings so its tail DMAs parallelize.
                for ci, (a, bnd, _regs) in enumerate(OUT_CHUNKS):
                    if h == 0:
                        oeng = nc.scalar
                    else:
                        oeng = nc.sync if ci % 2 == 0 else nc.scalar
                    oeng.dma_start(out=outs[h][:, a:bnd],
                                   in_=ot[:, a:bnd])
    nc.compile()
    return nc


def _get_nc():
    if "nc" not in _STATE:
        _STATE["nc"] = _build()
    return _STATE["nc"]


def _quantize(x):
    import ml_dtypes
    return x.astype(np.float16).astype(ml_dtypes.float8_e4m3)


def _prep_half(x8h):
    """x8h: (128, 128, 768) fp8 for one half -> per-core [128, 8, 2, 768]."""
    out = []
    for c in range(NCORES):
        blk = x8h[NB * c:NB * (c + 1)]                    # (16, 128, 768)
        # b = 2*td + s, n = p  ->  (p, td, s, f)
        out.append(np.ascontiguousarray(
            blk.reshape(TD, 2, P, D).transpose(2, 0, 1, 3)))
    return out


def kernel(x, label=None, genre_label=None, _trace=False):
    from concourse.bass_utils import run_bass_kernel_spmd

    nc = _get_nc()

    x = np.asarray(x, dtype=np.float32)
    x8 = [_quantize(x[0::2]), _quantize(x[1::2])]
    halves = [_prep_half(x8[0]), _prep_half(x8[1])]
    in_maps = [{"x0": halves[0][c], "x1": halves[1][c]} for c in range(NCORES)]

    # First execution of a freshly compiled NEFF has been observed to be
    # flaky (garbage output or device error); validate and retry.
    res = None
    for attempt in range(3):
        try:
            res = run_bass_kernel_spmd(nc, in_maps, list(range(NCORES)),
                                       trace=_trace)
        except Exception:
            if attempt == 2:
                raise
            continue
        ok = all(
            np.isfinite(np.asarray(res.results[c][f"o{h}"],
                                   dtype=np.float32)).all()
            and np.any(np.asarray(res.results[c][f"o{h}"], dtype=np.float32))
            for c in range(NCORES) for h in range(2))
        if ok:
            break
    LAST["res"] = res

    B = x.shape[0] // 2
    N = x.shape[1]
    tol = B * N

    loss = 0.0
    for h in range(2):
        U = np.zeros((D, D), dtype=np.float64)
        for c in range(NCORES):
            o = np.asarray(res.results[c][f"o{h}"], dtype=np.float64)
            for (name, i, c0, w, oc) in REGIONS:
                U[128 * i:128 * (i + 1), c0:c0 + w] += o[:, oc:oc + w]
        G = np.zeros((D, D), dtype=np.float64)
        for i in range(6):
            ri = slice(P * i, P * (i + 1))
            G[ri, P * i:D] = U[ri, P * i:D]
            for j in range(i + 1, 6):
                rj = slice(P * j, P * (j + 1))
                G[rj, ri] = U[ri, rj].T
        # row sums from the same quantized data (consistent with G)
        S = x8[h].astype(np.float64).sum(axis=1)          # (B, D)
        xbar = S / N
        M = xbar.T @ xbar
        mean = xbar.mean(axis=0)
        within = (G - N * M) / tol
        between = N * (M - B * np.outer(mean, mean)) / tol
        w_h = within / np.sqrt(np.sum(np.diagonal(within) ** 2))
        b_h = between / np.sqrt(np.sum(np.diagonal(between) ** 2))
        if h == 0:
            w0, b0 = w_h, b_h
        else:
            loss = np.sum((w0 - w_h) ** 2) + np.sum((b0 - b_h) ** 2)
    return np.asarray(loss, dtype=np.float32)


# revision 13
# speedup vs baseline: 1.3912x; 1.0149x over previous
"""Trainium2 Bass kernel for nn_LossFunction_40346922778857 (v13).

Computes: scatter-loss over x (256,128,768).
  x1 = x[::2], x2 = x[1::2]  (each (128,128,768))
  per half: within (D,D), between (D,D) scatter matrices, corr-normalized,
  loss = sum((w1-w2)^2) + sum((b1-b2)^2).

Device computes per-half Gram G = X^T X (upper-triangle 128-row blocks) in
fp8e4 DoubleRow; per-b row sums S are computed on host from the SAME fp8
data (keeps within/between consistent with the quantized Gram).  Host sums
the 8 cores' partials and finishes the O(D^2) algebra in float64.

Layout (per core):
  in  x{h}: [128, 8(td), 2(s), 768] fp8, row (td*256+s*128+p) feature f
  out o{h}: [128, 2688] bf16, regions
      [i0a 512][i0b 256][i1a 512][i1b 128][i2 512][i3 384][i4 256][i5 128]
      region (i, colrange): G rows 128i:128(i+1), cols below.
"""

import numpy as np

P = 128
D = 768
NB = 16          # b's per half per core
TD = 8           # double-k-tiles per half (each contracts 256 rows)
NCORES = 8
OW = 2688        # output cols per half

_STATE = {}
LAST = {}

# (name, block i, col offset in G row-block, width, out col offset)
REGIONS = [
    ("i0a", 0, 0,   512, 0),
    ("i0b", 0, 512, 256, 512),
    ("i1a", 1, 128, 512, 768),
    ("i1b", 1, 640, 128, 1280),
    ("i2",  2, 256, 512, 1408),
    ("i3",  3, 384, 384, 1920),
    ("i4",  4, 512, 256, 2304),
    ("i5",  5, 640, 128, 2560),
]  # c0 = absolute G column start

# PSUM bank map per half: region -> (bank tile idx, col offset in bank)
# h0 packs into 6 banks (b1 and b5 shared) so h1 starts with b6/b7 free;
# within each td, emission order guarantees the bank-sharing start=True
# owner (i0b before i1b, i4 before i5) writes first.
# h1 uses 8 distinct banks, each fully ordered after its h0 readers via
# AP overlap (512-wide regions land on h0's shared banks so the whole-bank
# clear of start=True is covered by WAR deps on both h0 casts).
BANKS = [
    {"i0a": (0, 0), "i0b": (1, 0), "i1a": (2, 0), "i1b": (1, 256),
     "i2": (3, 0), "i3": (4, 0), "i4": (5, 0), "i5": (5, 256)},
    {"i0a": (7, 0), "i0b": (6, 0), "i1a": (1, 0), "i1b": (0, 0),
     "i2": (5, 0), "i3": (2, 0), "i4": (3, 0), "i5": (4, 0)},
]

# drain emission order + engine (v=DVE, a=ACT): ordered so the banks h1
# needs earliest are freed first on each engine.
DRAINS = [("i0a", "v"), ("i0b", "a"), ("i1b", "a"), ("i1a", "a"),
          ("i2", "v"), ("i3", "a"), ("i4", "v"), ("i5", "v")]

# output DMAs per half: (out col start, out col end, regions covered)
OUT_CHUNKS = [
    (0, 768, ("i0a", "i0b")),
    (768, 1408, ("i1a", "i1b")),
    (1408, 2304, ("i2", "i3")),
    (2304, 2688, ("i4", "i5")),
]


def _build():
    import concourse.tile as tile
    from concourse import bacc, mybir

    # (walrus --enable-ldw-opt=true was tried to dedup the i0a/i0b and
    # i1a/i1b shared weight loads, but its standalone-InstLdweights output
    # crashes CoreV3 codegen in this compiler build.)
    nc = bacc.Bacc("TRN2", target_bir_lowering=False, debug=False,
                   num_devices=NCORES)

    fp8 = mybir.dt.float8e4
    xins = [nc.dram_tensor(f"x{h}", [P, TD, 2, D], fp8,
                           kind="ExternalInput").ap() for h in range(2)]
    outs = [nc.dram_tensor(f"o{h}", [P, OW], mybir.dt.bfloat16,
                           kind="ExternalOutput").ap() for h in range(2)]

    with tile.TileContext(nc) as tc:
        with tc.tile_pool(name="xp", bufs=2) as xp, \
             tc.tile_pool(name="wp", bufs=1) as wp, \
             tc.tile_pool(name="pp", bufs=8, space="PSUM") as pp, \
             tc.tile_pool(name="op", bufs=2) as op:
            xts = [xp.tile([P, TD, 2, D], fp8, tag="xt", name=f"xt{h}")
                   for h in range(2)]
            ots = [op.tile([P, OW], mybir.dt.bfloat16, tag="ot",
                           name=f"ot{h}") for h in range(2)]
            banks = [pp.tile([P, 512], mybir.dt.float32, tag="ps",
                             name=f"bank{b}") for b in range(8)]
            wt = wp.tile([P, 640], mybir.dt.float16, tag="wt")

            # input DMAs: (4 td x partition-band) chunks — 6144B contiguous
            # per-partition runs; both HWDGE rings carry part of every td
            # window so arrival tracks consumption order.  The sync ring
            # gets a bigger band (72/56) because the scalar ring starts
            # ~1.4us later.  (A third gpsimd/SWDGE queue was tried: ~4us
            # latency + descriptor-gen cost made everything worse.)
            # h0's chunks mostly on the sync ring (it starts ~1.4us
            # earlier), h1's mostly on scalar: h0's last chunk lands
            # ~1.2us sooner, and h1's data is all present well before
            # the PE reaches it.
            for eng, h, tdr in ((nc.sync, 0, slice(0, 2)),
                                (nc.scalar, 0, slice(2, 4)),
                                (nc.sync, 0, slice(4, 6)),
                                (nc.sync, 0, slice(6, 8)),
                                (nc.scalar, 1, slice(0, 2)),
                                (nc.scalar, 1, slice(2, 4)),
                                (nc.scalar, 1, slice(4, 6)),
                                (nc.sync, 1, slice(6, 8))):
                eng.dma_start(out=xts[h][:, tdr, :, :],
                              in_=xins[h][:, tdr, :, :])

            # PE warm-up on zeros while inputs stream: ~3.4us of activity
            # brings the HAM clock gate to 8/8 as real matmuls start.
            # memset on gpsimd: its stream is free at t=0.
            nc.gpsimd.memset(wt[:], 0.0)
            for _ in range(8):
                nc.tensor.matmul(banks[6][:, :512], wt[:, :128],
                                 wt[:, 128:640], start=True, stop=True)

            rdict = {r[0]: r for r in REGIONS}

            def mm(h, td, name, st, sp):
                xt = xts[h]
                (_n, i, c0, w, _oc) = rdict[name]
                b, boff = BANKS[h][name]
                nc.tensor.matmul(banks[b][:, boff:boff + w],
                                 xt[:, td, :, 128 * i:128 * i + 128],
                                 xt[:, td, :, c0:c0 + w],
                                 start=st, stop=sp, skip_group_check=True,
                                 perf_mode=mybir.MatmulPerfMode.DoubleRow)

            for h in range(2):
                bmap = BANKS[h]
                # td-major over td0-5 (streams with input arrival); the
                # bank-sharing first writer (h0) uses start=True at td0
                # (start clears the whole 2KB bank incl. the co-located
                # region).
                for td in range(TD - 4):
                    for (name, i, c0, w, _oc) in REGIONS:
                        mm(h, td, name, (td == 0) and bmap[name][1] == 0,
                           False)
                # region-PAIR-major over the last four tds: pairs of
                # regions complete ~1.2us apart so PSUM drains and output
                # DMAs pipeline with the compute tail, while consecutive
                # matmuls alternate banks (no same-bank accumulate
                # serialization on the PE write port).
                dn = [n for n, _e in DRAINS]
                for pa, pb in ((dn[0], dn[1]), (dn[2], dn[3]),
                               (dn[4], dn[5]), (dn[6], dn[7])):
                    for td in range(TD - 4, TD):
                        mm(h, td, pa, False, td == TD - 1)
                        mm(h, td, pb, False, td == TD - 1)
                # drains: PSUM -> SBUF bf16 split across DVE + ACT
                ot = ots[h]
                for name, e in DRAINS:
                    (_n, i, c0, w, oc) = rdict[name]
                    b, boff = bmap[name]
                    src = banks[b][:, boff:boff + w]
                    dst = ot[:, oc:oc + w]
                    if e == "v":
                        nc.vector.tensor_copy(dst, src)
                    else:
                        nc.scalar.copy(dst, src)
                # outputs stream per chunk; h0 on the scalar ring, h1
                # split across both rings so its tail DMAs parallelize.
                for ci, (a, bnd, _regs) in enumerate(OUT_CHUNKS):
                    if h == 0:
                        oeng = nc.scalar
                    else:
                        oeng = nc.sync if ci % 2 == 0 else nc.scalar
                    oeng.dma_start(out=outs[h][:, a:bnd],
                                   in_=ot[:, a:bnd])
    nc.compile()
    return nc


def _get_nc():
    if "nc" not in _STATE:
        _STATE["nc"] = _build()
    return _STATE["nc"]


def _quantize(x):
    import ml_dtypes
    return x.astype(np.float16).astype(ml_dtypes.float8_e4m3)


def _prep_half(x8h):
    """x8h: (128, 128, 768) fp8 for one half -> per-core [128, 8, 2, 768]."""
    out = []
    for c in range(NCORES):
        blk = x8h[NB * c:NB * (c + 1)]                    # (16, 128, 768)
        # b = 2*td + s, n = p  ->  (p, td, s, f)
        out.append(np.ascontiguousarray(
            blk.reshape(TD, 2, P, D).transpose(2, 0, 1, 3)))
    return out


def kernel(x, label=None, genre_label=None, _trace=False):
    from concourse.bass_utils import run_bass_kernel_spmd

    nc = _get_nc()

    x = np.asarray(x, dtype=np.float32)
    x8 = [_quantize(x[0::2]), _quantize(x[1::2])]
    halves = [_prep_half(x8[0]), _prep_half(x8[1])]
    in_maps = [{"x0": halves[0][c], "x1": halves[1][c]} for c in range(NCORES)]

    # First execution of a freshly compiled NEFF has been observed to be
    # flaky (garbage output or device error); validate and retry.
    res = None
    for attempt in range(3):
        try:
            res = run_bass_kernel_spmd(nc, in_maps, list(range(NCORES)),
                                       trace=_trace)
        except Exception:
            if attempt == 2:
                raise
            continue
        ok = all(
            np.isfinite(np.asarray(res.results[c][f"o{h}"],
                                   dtype=np.float32)).all()
            and np.any(np.asarray(res.results[c][f"o{h}"], dtype=np.float32))
            for c in range(NCORES) for h in range(2))
        if ok:
            break
    LAST["res"] = res

    B = x.shape[0] // 2
    N = x.shape[1]
    tol = B * N

    loss = 0.0
    for h in range(2):
        U = np.zeros((D, D), dtype=np.float64)
        for c in range(NCORES):
            o = np.asarray(res.results[c][f"o{h}"], dtype=np.float64)
            for (name, i, c0, w, oc) in REGIONS:
                U[128 * i:128 * (i + 1), c0:c0 + w] += o[:, oc:oc + w]
        G = np.zeros((D, D), dtype=np.float64)
        for i in range(6):
            ri = slice(P * i, P * (i + 1))
            G[ri, P * i:D] = U[ri, P * i:D]
            for j in range(i + 1, 6):
                rj = slice(P * j, P * (j + 1))
                G[rj, ri] = U[ri, rj].T
        # row sums from the same quantized data (consistent with G)
        S = x8[h].astype(np.float64).sum(axis=1)          # (B, D)
        xbar = S / N
        M = xbar.T @ xbar
        mean = xbar.mean(axis=0)
        within = (G - N * M) / tol
        between = N * (M - B * np.outer(mean, mean)) / tol
        w_h = within / np.sqrt(np.sum(np.diagonal(within) ** 2))
        b_h = between / np.sqrt(np.sum(np.diagonal(between) ** 2))
        if h == 0:
            w0, b0 = w_h, b_h
        else:
            loss = np.sum((w0 - w_h) ** 2) + np.sum((b0 - b_h) ** 2)
    return np.asarray(loss, dtype=np.float32)
